# revision 1
# baseline (speedup 1.0000x reference)
"""Trainium2 Bass kernel for nn_BlockModel_82678120448388.

Model: per (batch, head): 8x8 transition matrices from an MLP (normalized),
values from a second MLP, then a linear recurrence s_t = A_t s_{t-1} + v_t
over seq=2048.

Sharding: 8 cores = 4 batches x 2 head-halves (32 heads each). Weights
replicated / row-sliced on host; full inputs in, full output out.
"""

import numpy as np
import ml_dtypes
from contextlib import ExitStack

import concourse.bass as bass
import concourse.bacc as bacc
import concourse.tile as tile
from concourse import mybir

F32 = mybir.dt.float32
BF16 = mybir.dt.bfloat16
AF = mybir.ActivationFunctionType
ALU = mybir.AluOpType

BS, SEQ, EMB, BD = 4, 2048, 512, 8
H = EMB // BD      # 64 global heads
HL = 32            # heads per core
NF = HL * BD * BD  # 2048 blk feats per core
VF = HL * BD       # 256 v feats per core
HID = EMB * BD     # 4096
P = 128
JW = BD + 1        # augmented [T|u] column count

N_CORES = 8


def build_nc(TOK=SEQ, K=16, p1_steps=None, pc_steps=None, nq_steps=None):
    """Per-core Bass module. TOK tokens, K chunks (chunk len C=TOK//K)."""
    C = TOK // K
    QT = min(512, TOK)     # L1 token-chunk
    NQ = TOK // QT
    TPQ = QT // P          # tok-tiles per q
    NHO = P // K           # head-groups per chunk on partitions (8 for K=16)
    NHR = HL // NHO        # heads per group in free dim (4)
    HRI = NHR * BD         # 32

    assert TOK % QT == 0 and QT % P == 0 and P % K == 0

    nc = bacc.Bacc("TRN2", target_bir_lowering=False, debug=False)

    xT = nc.dram_tensor("xT", [EMB, TOK], BF16, kind="ExternalInput")
    w1 = nc.dram_tensor("w1", [EMB, HID], BF16, kind="ExternalInput")
    b1 = nc.dram_tensor("b1", [HID, 1], F32, kind="ExternalInput")
    w2 = nc.dram_tensor("w2", [HID, NF], BF16, kind="ExternalInput")
    b2 = nc.dram_tensor("b2", [1, NF], BF16, kind="ExternalInput")
    v1 = nc.dram_tensor("v1", [EMB, EMB], BF16, kind="ExternalInput")
    c1 = nc.dram_tensor("c1", [EMB, 1], F32, kind="ExternalInput")
    v2 = nc.dram_tensor("v2", [EMB, VF], BF16, kind="ExternalInput")
    c2 = nc.dram_tensor("c2", [1, VF], BF16, kind="ExternalInput")
    a0 = nc.dram_tensor("a0", [NHO, HRI], F32, kind="ExternalInput")
    smat = nc.dram_tensor("smat", [P, P], F32, kind="ExternalInput")
    tinit = nc.dram_tensor("tinit", [P, K * JW], F32, kind="ExternalInput")
    out = nc.dram_tensor("out", [TOK, VF], F32, kind="ExternalOutput")

    a_dram = nc.dram_tensor("a_scratch", [TOK, NF], F32)
    tst_dram = nc.dram_tensor("tst_scratch", [2 * P, K * JW], F32)
    v_dram = nc.dram_tensor("v_scratch", [TOK, VF], F32)

    with ExitStack() as ctx:
        tc = ctx.enter_context(tile.TileContext(nc))
        cpool = ctx.enter_context(tc.tile_pool(name="consts", bufs=1))
        wpool = ctx.enter_context(tc.tile_pool(name="weights", bufs=1))
        xpool = ctx.enter_context(tc.tile_pool(name="xstream", bufs=2))
        hpool = ctx.enter_context(tc.tile_pool(name="hidden", bufs=1))
        w2pool = ctx.enter_context(tc.tile_pool(name="w2stream", bufs=4))
        l1ps = ctx.enter_context(tc.tile_pool(name="l1ps", bufs=2, space="PSUM"))
        p1ps = ctx.enter_context(tc.tile_pool(name="p1ps", bufs=2, space="PSUM"))
        l2ps = ctx.enter_context(tc.tile_pool(name="l2ps", bufs=TPQ, space="PSUM"))
        vps = ctx.enter_context(tc.tile_pool(name="vps", bufs=1, space="PSUM"))
        blkpool = ctx.enter_context(tc.tile_pool(name="blk", bufs=TPQ + 1))
        pwpool = ctx.enter_context(tc.tile_pool(name="pw", bufs=2))
        smpool = ctx.enter_context(tc.tile_pool(name="small", bufs=3))
        vtpool = ctx.enter_context(tc.tile_pool(name="vtile", bufs=2))
        agpool = ctx.enter_context(tc.tile_pool(name="agather", bufs=3))
        vgpool = ctx.enter_context(tc.tile_pool(name="vgather", bufs=3))
        mopool = ctx.enter_context(tc.tile_pool(name="multout", bufs=3))
        tupool = ctx.enter_context(tc.tile_pool(name="tu", bufs=2))
        scpool = ctx.enter_context(tc.tile_pool(name="scan", bufs=1))

        # ---- constants / weights ----
        ones_s = cpool.tile([1, P], BF16, tag="ones")
        nc.vector.memset(ones_s[:], 1.0)
        b1_s = cpool.tile([P, HID // P], F32, tag="b1")
        nc.sync.dma_start(b1_s[:], b1[:].rearrange("(m p) one -> p (m one)", p=P))
        c1_s = cpool.tile([P, EMB // P], F32, tag="c1")
        nc.sync.dma_start(c1_s[:], c1[:].rearrange("(m p) one -> p (m one)", p=P))
        b2_s = cpool.tile([1, NF], BF16, tag="b2")
        nc.sync.dma_start(b2_s[:], b2[:])
        c2_s = cpool.tile([1, VF], BF16, tag="c2")
        nc.sync.dma_start(c2_s[:], c2[:])
        a0_s = cpool.tile([NHO, HRI], F32, tag="a0")
        nc.sync.dma_start(a0_s[:], a0[:])
        smat_s = cpool.tile([P, P], F32, tag="smat")
        nc.sync.dma_start(smat_s[:], smat[:])

        v1_s = wpool.tile([P, 4, EMB], BF16, tag="v1")
        nc.sync.dma_start(v1_s[:], v1[:].rearrange("(k p) m -> p k m", p=P))
        v2_s = wpool.tile([P, 4, VF], BF16, tag="v2")
        nc.sync.dma_start(v2_s[:], v2[:].rearrange("(k p) n -> p k n", p=P))

        # ================= scan helpers =================
        # a_dram row tau*128 + c*8 + j holds token c*C + 8*tau + j, feats in
        # (head, col, row) order. Phase 1 layout: partition = (hpack16, k8),
        # Tst[(h,k), (c, j9)] = [T|u][row k, col j] for chunk c; two packs.
        TUP = NHR * BD * JW  # 288 (old layout, used by phase B/C)
        TSP = K * JW         # 144 Tst row size

        def rowbase(r):
            tau, j = r // 8, r % 8
            return tau * P + j

        tu_box = {}

        def g_A(r):
            ag = agpool.tile([P, HL * BD], F32, tag="ag", name=f"ag{r}")
            nc.sync.dma_start(ag[:], bass.AP(
                a_dram, rowbase(r) * NF,
                [[8 * NF, K], [NHR * BD * BD, NHO], [1, NHR * BD * BD]]))
            return ag

        def g_v(r):
            vg = vgpool.tile([P, HRI], F32, tag="vg", name=f"vg{r}")
            nc.sync.dma_start(vg[:], bass.AP(
                v_dram, rowbase(r) * VF,
                [[8 * VF, K], [NHR * BD, NHO], [1, HRI]]))
            return vg

        def phase1_init():
            tu = tupool.tile([P, TUP], F32, tag="tu", name="tu0")
            ag0, vg0 = g_A(0), g_v(0)
            # T := A_0 ; ag block content is (hr, col, row)
            nc.vector.tensor_copy(
                bass.AP(tu.tensor, tu[:].offset,
                        [[TUP, P], [BD * JW, NHR], [JW, BD], [1, BD]]),
                bass.AP(ag0.tensor, ag0[:].offset,
                        [[HL * BD, P], [BD * BD, NHR], [1, BD], [BD, BD]]))
            nc.vector.tensor_copy(
                bass.AP(tu.tensor, tu[:].offset + BD,
                        [[TUP, P], [BD * JW, NHR], [JW, BD]]),
                bass.AP(vg0.tensor, vg0[:].offset,
                        [[HRI, P], [BD, NHR], [1, BD]]))
            tu_box['tu'] = tu

        def phase1_step(r):
            tu = tu_box['tu']
            ag, vg = g_A(r), g_v(r)
            mo = mopool.tile([P, TUP * BD], F32, tag="mo", name=f"mo{r}")
            for hr in range(NHR):
                # out[i, j9, k8] = A[i, k] * Tu[k, j]; A elem (i,k) at k*8+i
                nc.vector.tensor_tensor(
                    bass.AP(mo.tensor, mo[:].offset + hr * BD * JW * BD,
                            [[TUP * BD, P], [JW * BD, BD], [BD, JW], [1, BD]]),
                    bass.AP(ag.tensor, ag[:].offset + hr * BD * BD,
                            [[HL * BD, P], [1, BD], [0, JW], [BD, BD]]),
                    bass.AP(tu.tensor, tu[:].offset + hr * BD * JW,
                            [[TUP, P], [0, BD], [1, JW], [JW, BD]]),
                    ALU.mult)
            tun = tupool.tile([P, TUP], F32, tag="tu", name=f"tu{r}")
            nc.vector.tensor_reduce(
                bass.AP(tun.tensor, tun[:].offset, [[TUP, P], [1, TUP]]),
                bass.AP(mo.tensor, mo[:].offset,
                        [[TUP * BD, P], [BD, TUP], [1, BD]]),
                axis=mybir.AxisListType.X, op=ALU.add)
            nc.vector.tensor_tensor(
                bass.AP(tun.tensor, tun[:].offset + BD,
                        [[TUP, P], [BD * JW, NHR], [JW, BD]]),
                bass.AP(tun.tensor, tun[:].offset + BD,
                        [[TUP, P], [BD * JW, NHR], [JW, BD]]),
                bass.AP(vg.tensor, vg[:].offset,
                        [[HRI, P], [BD, NHR], [1, BD]]),
                ALU.add)
            tu_box['tu'] = tun

        # ================= stage A (+ interleaved phase 1) =================
        for q in range(NQ if nq_steps is None else nq_steps):
            RPQ = TPQ * 8  # r-range covered by this q
            xq = xpool.tile([P, 4, QT], BF16, tag="xq")
            for ttq in range(TPQ):
                # tile tau = q*TPQ+ttq: tokens c*C + 8*tau + j, col order (c, j)
                for k in range(4):
                    nc.sync.dma_start(
                        xq[:, k, bass.ts(ttq, P)],
                        bass.AP(xT, k * P * TOK + q * RPQ + ttq * 8,
                                [[TOK, P], [C, K], [1, 8]]))

            hid_t = hpool.tile([P, HID // P, QT], BF16, tag="hid")
            for m in range(HID // P):
                w1m = w2pool.tile([P, 4, P], BF16, tag="w1m", name=f"w1m{q}_{m}")
                nc.sync.dma_start(
                    w1m[:], w1[:, bass.ts(m, P)].rearrange("(k p) m -> p k m", p=P))
                ps = l1ps.tile([P, QT], F32, tag="l1")
                for k in range(4):
                    nc.tensor.matmul(ps[:], w1m[:, k, :], xq[:, k, :],
                                     start=(k == 0), stop=(k == 3))
                nc.scalar.activation(hid_t[:, m, :], ps[:], AF.Relu,
                                     bias=b1_s[:, m:m + 1])

            hv_t = hpool.tile([P, 4, QT], BF16, tag="hv")
            for m in range(4):
                ps = l1ps.tile([P, QT], F32, tag="l1")
                for k in range(4):
                    nc.tensor.matmul(ps[:], v1_s[:, k, bass.ts(m, P)], xq[:, k, :],
                                     start=(k == 0), stop=(k == 3))
                nc.scalar.activation(hv_t[:, m, :], ps[:], AF.Relu,
                                     bias=c1_s[:, m:m + 1])

            # ---- L2: token-major blk, W2 streamed per (n, k) ----
            blks = [blkpool.tile([P, NF], F32, tag="blk", name=f"blk{q}_{i}") for i in range(TPQ)]
            for n in range(NF // 512):
                pss = [l2ps.tile([P, 512], F32, tag="l2", name=f"l2ps{q}_{n}_{i}") for i in range(TPQ)]
                for ttq in range(TPQ):
                    nc.tensor.matmul(pss[ttq][:], ones_s[:1, :],
                                     b2_s[:1, bass.ts(n, 512)], start=True, stop=False)
                for k in range(HID // P):
                    w2s = w2pool.tile([P, 512], BF16, tag="w2s")
                    nc.sync.dma_start(w2s[:], w2[bass.ts(k, P), bass.ts(n, 512)])
                    for ttq in range(TPQ):
                        nc.tensor.matmul(pss[ttq][:], hid_t[:, k, bass.ts(ttq, P)],
                                         w2s[:], start=False, stop=(k == HID // P - 1))
                for ttq in range(TPQ):
                    nc.scalar.activation(blks[ttq][:, bass.ts(n, 512)], pss[ttq][:],
                                         AF.Identity)

            # ---- v2 + normalization per tok-tile ----
            for ttq in range(TPQ):
                tt = q * TPQ + ttq
                rowsl = bass.ds(tt * P, P)

                psv = vps.tile([P, VF], F32, tag="v")
                nc.tensor.matmul(psv[:], ones_s[:1, :], c2_s[:1, :],
                                 start=True, stop=False)
                for k in range(4):
                    nc.tensor.matmul(psv[:], hv_t[:, k, bass.ts(ttq, P)],
                                     v2_s[:, k, :], start=False, stop=(k == 3))
                vt = vtpool.tile([P, VF], F32, tag="vt")
                nc.scalar.activation(vt[:], psv[:], AF.Identity)
                nc.sync.dma_start(v_dram[rowsl, :], vt[:])

                blk = blks[ttq]
                pw = pwpool.tile([P, NF], F32, tag="pw")
                nc.scalar.activation(pw[:], blk[:], AF.Square)
                nc.scalar.activation(pw[:], pw[:], AF.Ln)
                nc.scalar.activation(pw[:], pw[:], AF.Exp, scale=0.6)
                # sum over i: feat = h*64 + i*8 + j -> dims [p, h, j, i]
                pst = smpool.tile([P, HL * BD], F32, tag="pst")
                nc.vector.tensor_reduce(
                    pst[:].rearrange("p (h j) -> p h j", h=HL, j=BD),
                    bass.AP(pw.tensor, pw[:].offset,
                            [[NF, P], [64, HL], [1, BD], [8, BD]]),
                    axis=mybir.AxisListType.X, op=ALU.add)
                nc.scalar.activation(pst[:], pst[:], AF.Ln)
                nc.scalar.activation(pst[:], pst[:], AF.Exp, scale=1.0 / 1.2)
                dm = smpool.tile([P, HL], F32, tag="dm")
                nc.vector.tensor_reduce(
                    dm[:].rearrange("p (h one) -> p h one", h=HL, one=1),
                    pst[:].rearrange("p (h j) -> p h j", h=HL, j=BD),
                    axis=mybir.AxisListType.X, op=ALU.max)
                rc = smpool.tile([P, HL], F32, tag="rc")
                nc.vector.reciprocal(rc[:], dm[:])
                # A = blk * rc (broadcast over i, j) -> into pw buffer
                # write A transposed per head: feat order (h, col j, row i)
                nc.vector.tensor_tensor(
                    bass.AP(pw.tensor, pw[:].offset,
                            [[NF, P], [64, HL], [1, BD], [8, BD]]),
                    blk[:].rearrange("p (h i j) -> p h i j", h=HL, i=BD, j=BD),
                    bass.AP(rc.tensor, rc[:].offset,
                            [[HL, P], [1, HL], [0, BD], [0, BD]]),
                    ALU.mult)
                nc.sync.dma_start(a_dram[rowsl, :], pw[:])

            # ---- phase 1 steps for this q's token tiles ----
            RPQ_ = TPQ * 8
            for r in range(q * RPQ_, (q + 1) * RPQ_):
                if p1_steps is not None and r >= p1_steps:
                    continue
                if r == 0:
                    phase1_init()
                else:
                    phase1_step(r)

        # ---- phase B: chunk-level combine (on partitions 0:NHO) ----
        tu = tu_box['tu']
        TUPK = K * TUP
        tu2 = scpool.tile([NHO, TUPK], F32, tag="tu2")
        for c in range(K):
            nc.sync.dma_start(tu2[:, c * TUP:(c + 1) * TUP],
                              tu[c * NHO:(c + 1) * NHO, :])
        s_seq = scpool.tile([NHO, (K + 1) * HRI], F32, tag="sseq")
        nc.vector.tensor_copy(s_seq[:, 0:HRI], a0_s[:])
        for c in range(K):
            mo3 = mopool.tile([NHO, HRI * BD], F32, tag="mo3")
            nc.vector.tensor_tensor(
                bass.AP(mo3.tensor, mo3[:].offset,
                        [[HRI * BD, NHO], [BD * BD, NHR], [BD, BD], [1, BD]]),
                bass.AP(tu2.tensor, tu2[:].offset + c * TUP,
                        [[TUPK, NHO], [BD * JW, NHR], [JW, BD], [1, BD]]),
                bass.AP(s_seq.tensor, s_seq[:].offset + c * HRI,
                        [[(K + 1) * HRI, NHO], [BD, NHR], [0, BD], [1, BD]]),
                ALU.mult)
            sn3 = smpool.tile([NHO, HRI], F32, tag="sn3")
            nc.vector.tensor_reduce(
                bass.AP(sn3.tensor, sn3[:].offset, [[HRI, NHO], [1, HRI]]),
                bass.AP(mo3.tensor, mo3[:].offset,
                        [[HRI * BD, NHO], [BD, HRI], [1, BD]]),
                axis=mybir.AxisListType.X, op=ALU.add)
            nc.vector.tensor_tensor(
                bass.AP(s_seq.tensor, s_seq[:].offset + (c + 1) * HRI,
                        [[(K + 1) * HRI, NHO], [BD, NHR], [1, BD]]),
                bass.AP(sn3.tensor, sn3[:].offset, [[HRI, NHO], [BD, NHR], [1, BD]]),
                bass.AP(tu2.tensor, tu2[:].offset + c * TUP + BD,
                        [[TUPK, NHO], [BD * JW, NHR], [JW, BD]]),
                ALU.add)
        # relayout chunk-start states -> s_init [(c,ho), (hr,i)]
        s_init = scpool.tile([P, HRI], F32, tag="sinit")
        for c in range(K):
            nc.sync.dma_start(s_init[c * NHO:(c + 1) * NHO, :],
                              s_seq[:, c * HRI:(c + 1) * HRI])

        # ---- phase C: re-run with true init ----
        def gather_A(r):
            ag = agpool.tile([P, HL * BD], F32, tag="agc", name=f"agc{r}")
            nc.sync.dma_start(ag[:], bass.AP(
                a_dram, rowbase(r) * NF,
                [[8 * NF, K], [NHR * BD * BD, NHO], [1, NHR * BD * BD]]))
            return ag

        def gather_v(r):
            vg = vgpool.tile([P, HRI], F32, tag="vgc", name=f"vgc{r}")
            nc.sync.dma_start(vg[:], bass.AP(
                v_dram, rowbase(r) * VF,
                [[8 * VF, K], [NHR * BD, NHO], [1, HRI]]))
            return vg

        s_out = scpool.tile([P, C * HRI], F32, tag="sout")
        for r in range(C if pc_steps is None else pc_steps):
            ag, vg = gather_A(r), gather_v(r)
            sprev = (bass.AP(s_init.tensor, s_init[:].offset,
                             [[HRI, P], [BD, NHR], [0, BD], [1, BD]])
                     if r == 0 else
                     bass.AP(s_out.tensor, s_out[:].offset + (r - 1) * HRI,
                             [[C * HRI, P], [BD, NHR], [0, BD], [1, BD]]))
            mo2 = mopool.tile([P, HRI * BD], F32, tag="mo2")
            nc.vector.tensor_tensor(
                bass.AP(mo2.tensor, mo2[:].offset,
                        [[HRI * BD, P], [BD * BD, NHR], [BD, BD], [1, BD]]),
                bass.AP(ag.tensor, ag[:].offset,
                        [[HL * BD, P], [BD * BD, NHR], [1, BD], [BD, BD]]),
                sprev, ALU.mult)
            sred = smpool.tile([P, HRI], F32, tag="sred")
            nc.vector.tensor_reduce(
                bass.AP(sred.tensor, sred[:].offset, [[HRI, P], [1, HRI]]),
                bass.AP(mo2.tensor, mo2[:].offset,
                        [[HRI * BD, P], [BD, HRI], [1, BD]]),
                axis=mybir.AxisListType.X, op=ALU.add)
            nc.vector.tensor_tensor(
                bass.AP(s_out.tensor, s_out[:].offset + r * HRI,
                        [[C * HRI, P], [1, HRI]]),
                bass.AP(sred.tensor, sred[:].offset, [[HRI, P], [1, HRI]]),
                bass.AP(vg.tensor, vg[:].offset, [[HRI, P], [1, HRI]]),
                ALU.add)

        # ---- output: s_out [(c,ho), (r, hr, i)] -> out [t, vf] ----
        for c in range(K):
            nc.sync.dma_start(
                bass.AP(out, c * C * VF, [[HRI, NHO], [VF, C], [1, HRI]]),
                bass.AP(s_out.tensor, s_out[c * NHO:(c + 1) * NHO, :].offset,
                        [[C * HRI, NHO], [HRI, C], [1, HRI]]))

    nc.compile()
    return nc


# ---------------- host side ----------------

_NC_CACHE = {}


def _get_nc(TOK=SEQ, K=16):
    key = (TOK, K)
    if key not in _NC_CACHE:
        _NC_CACHE[key] = build_nc(TOK=TOK, K=K)
    return _NC_CACHE[key]


def prep_shared(W1, b1, W2, b2, V1, c1, V2, c2, a0):
    bf = ml_dtypes.bfloat16
    W2r = W2.reshape(H, BD, BD, HID)
    W2c = (W2r - W2r.mean(axis=1, keepdims=True)).reshape(H * BD * BD, HID)
    b2r = b2.reshape(H, BD, BD)
    b2c = (b2r - b2r.mean(axis=1, keepdims=True)).reshape(-1)
    shared = {
        "smat": np.kron(np.eye(16, dtype=np.float32),
                        np.ones((BD, BD), np.float32)),
        "tinit": np.tile(np.concatenate([np.eye(BD, dtype=np.float32),
                                         np.zeros((BD, 1), np.float32)], 1)
                         .reshape(BD, 1, 9), (16, 16, 1)).reshape(128, -1),
        "w1": np.ascontiguousarray(W1.T).astype(bf),
        "b1": np.asarray(b1).reshape(HID, 1).astype(np.float32),
        "v1": np.ascontiguousarray(V1.T).astype(bf),
        "c1": np.asarray(c1).reshape(EMB, 1).astype(np.float32),
    }
    halves = []
    for half in range(2):
        rsl = slice(half * NF, (half + 1) * NF)
        vsl = slice(half * VF, (half + 1) * VF)
        hsl = slice(half * HL, (half + 1) * HL)
        a0h = np.asarray(a0)[0, hsl]                       # [32, 8]
        a0p = a0h.reshape(BD, 4, BD).reshape(BD, 32)       # [ho, (hr, i)]
        halves.append({
            "w2": np.ascontiguousarray(W2c[rsl].T).astype(bf),
            "b2": b2c[rsl].reshape(1, NF).astype(bf),
            "v2": np.ascontiguousarray(V2[vsl].T).astype(bf),
            "c2": np.asarray(c2)[vsl].reshape(1, VF).astype(bf),
            "a0": a0p.astype(np.float32),
        })
    return shared, halves


def make_in_maps(x, W1, b1, W2, b2, V1, c1, V2, c2, a0):
    shared, halves = prep_shared(W1, b1, W2, b2, V1, c1, V2, c2, a0)
    bf = ml_dtypes.bfloat16
    in_maps = []
    for core in range(N_CORES):
        b, half = core // 2, core % 2
        m = dict(shared)
        m.update(halves[half])
        m["xT"] = np.ascontiguousarray(np.asarray(x)[b].T).astype(bf)
        in_maps.append(m)
    return in_maps


def kernel(x, W1, b1, W2, b2, V1, c1, V2, c2, a0):
    from concourse import bass_utils
    nc = _get_nc(SEQ)
    in_maps = make_in_maps(x, W1, b1, W2, b2, V1, c1, V2, c2, a0)
    res = bass_utils.run_bass_kernel_spmd(nc, in_maps, core_ids=list(range(N_CORES)))
    out = np.zeros((BS, SEQ, EMB), np.float32)
    for core in range(N_CORES):
        b, half = core // 2, core % 2
        out[b, :, half * VF:(half + 1) * VF] = res.results[core]["out"]
    return out



# revision 18
# speedup vs baseline: 1.2005x; 1.2005x over previous
"""Trainium2 Bass kernel for nn_BlockModel_82678120448388.

Model: per (batch, head): 8x8 transition matrices from an MLP (normalized),
values from a second MLP, then a linear recurrence s_t = A_t s_{t-1} + v_t
over seq=2048.

Sharding: 8 cores = 4 batches x 2 head-halves (32 heads each). Weights
replicated / row-sliced on host; full inputs in, full output out.

Scan strategy: 16 chunks of 128 tokens scanned in parallel across
partitions (partition = (chunk, head-group)). Phase 1 maintains the
augmented [T|u] prefix per chunk in fp16 with packed innermost-stride-1
layouts (2x DVE mode). Phase B combines chunk transitions sequentially
on 8 partitions. Phase C re-scans states with true chunk-start inits.
"""

import numpy as np
import ml_dtypes
from contextlib import ExitStack

import concourse.bass as bass
import concourse.bacc as bacc
import concourse.tile as tile
from concourse import mybir

F32 = mybir.dt.float32
BF16 = mybir.dt.bfloat16
FP16 = mybir.dt.float16
AF = mybir.ActivationFunctionType
ALU = mybir.AluOpType
AX = mybir.AxisListType

BS, SEQ, EMB, BD = 4, 2048, 512, 8
H = EMB // BD
HL = 32            # heads per core
NF = HL * BD * BD  # 2048 blk feats per core
VF = HL * BD       # 256 v feats per core
HID = EMB * BD     # 4096
P = 128
JW = BD + 1        # [T|u] column count (9)
K = 16             # chunks
NHO = P // K       # head-groups per chunk on partitions (8)
NHR = HL // NHO    # heads per group (4)
TUP = NHR * JW * BD  # 288 tu feats per partition
N_CORES = 8


def build_nc(TOK=SEQ):
    C = TOK // K           # tokens per chunk (128)
    QT = min(512, TOK)
    NQ = TOK // QT
    TPQ = QT // P          # token tiles per q (4)
    NT = TOK // P          # total token tiles (16)
    assert TOK % QT == 0 and QT % P == 0

    nc = bacc.Bacc("TRN2", target_bir_lowering=False, debug=False)

    xT = nc.dram_tensor("xT", [EMB, TOK], BF16, kind="ExternalInput")
    w1 = nc.dram_tensor("w1", [EMB, HID], BF16, kind="ExternalInput")
    b1 = nc.dram_tensor("b1", [HID, 1], F32, kind="ExternalInput")
    w2 = nc.dram_tensor("w2", [HID, NF], BF16, kind="ExternalInput")
    b2 = nc.dram_tensor("b2", [1, NF], BF16, kind="ExternalInput")
    v1 = nc.dram_tensor("v1", [EMB, EMB], BF16, kind="ExternalInput")
    c1 = nc.dram_tensor("c1", [EMB, 1], F32, kind="ExternalInput")
    v2 = nc.dram_tensor("v2", [EMB, VF], BF16, kind="ExternalInput")
    c2 = nc.dram_tensor("c2", [1, VF], BF16, kind="ExternalInput")
    a0 = nc.dram_tensor("a0", [NHO, NHR * BD], FP16, kind="ExternalInput")
    out = nc.dram_tensor("out", [TOK, VF], F32, kind="ExternalOutput")

    # scratch: row (tau*128 + c*8 + j) = token c*C + 8*tau + j
    a_dram = nc.dram_tensor("a_scratch", [TOK, NF], FP16)
    v_dram = nc.dram_tensor("v_scratch", [TOK, VF], FP16)

    HRI = NHR * BD  # 32

    with ExitStack() as ctx:
        tc = ctx.enter_context(tile.TileContext(nc))
        cpool = ctx.enter_context(tc.tile_pool(name="consts", bufs=1))
        wpool = ctx.enter_context(tc.tile_pool(name="weights", bufs=1))
        xpool = ctx.enter_context(tc.tile_pool(name="xstream", bufs=2))
        w1pool = ctx.enter_context(tc.tile_pool(name="w1s", bufs=2))
        hpool = ctx.enter_context(tc.tile_pool(name="hidden", bufs=1))
        w2pool = ctx.enter_context(tc.tile_pool(name="w2s", bufs=3))
        l1ps = ctx.enter_context(tc.tile_pool(name="l1ps", bufs=2, space="PSUM"))
        l2ps = ctx.enter_context(tc.tile_pool(name="l2ps", bufs=TPQ, space="PSUM"))
        vps = ctx.enter_context(tc.tile_pool(name="vps", bufs=1, space="PSUM"))
        blkpool = ctx.enter_context(tc.tile_pool(name="blk", bufs=TPQ + 1))
        sqpool = ctx.enter_context(tc.tile_pool(name="sq", bufs=3))
        smpool = ctx.enter_context(tc.tile_pool(name="small", bufs=6))
        apool = ctx.enter_context(tc.tile_pool(name="aT", bufs=2))
        vtpool = ctx.enter_context(tc.tile_pool(name="vT", bufs=2))
        agpool = ctx.enter_context(tc.tile_pool(name="ag", bufs=2))
        vgpool = ctx.enter_context(tc.tile_pool(name="vg", bufs=NT))
        tupool = ctx.enter_context(tc.tile_pool(name="tu", bufs=2))
        mopool = ctx.enter_context(tc.tile_pool(name="mo", bufs=2))
        s1pool = ctx.enter_context(tc.tile_pool(name="s1", bufs=2))
        s2pool = ctx.enter_context(tc.tile_pool(name="s2", bufs=2))
        scpool = ctx.enter_context(tc.tile_pool(name="scan", bufs=1))
        pcpool = ctx.enter_context(tc.tile_pool(name="pc", bufs=2))

        # ---- constants / weights ----
        ones_s = cpool.tile([1, P], BF16, tag="ones")
        nc.vector.memset(ones_s[:], 1.0)
        b1_s = cpool.tile([P, HID // P], F32, tag="b1")
        nc.sync.dma_start(b1_s[:], b1[:].rearrange("(m p) one -> p (m one)", p=P))
        c1_s = cpool.tile([P, EMB // P], F32, tag="c1")
        nc.sync.dma_start(c1_s[:], c1[:].rearrange("(m p) one -> p (m one)", p=P))
        b2_s = cpool.tile([1, NF], BF16, tag="b2")
        nc.sync.dma_start(b2_s[:], b2[:])
        c2_s = cpool.tile([1, VF], BF16, tag="c2")
        nc.sync.dma_start(c2_s[:], c2[:])
        a0_s = cpool.tile([NHO, HRI], FP16, tag="a0")
        nc.sync.dma_start(a0_s[:], a0[:])

        v1_s = wpool.tile([P, 4, EMB], BF16, tag="v1")
        nc.sync.dma_start(v1_s[:], v1[:].rearrange("(k p) m -> p k m", p=P))
        v2_s = wpool.tile([P, 4, VF], BF16, tag="v2")
        nc.sync.dma_start(v2_s[:], v2[:].rearrange("(k p) n -> p k n", p=P))

        ctx.enter_context(nc.allow_low_precision(reason="fp16 scan state"))

        vg_tiles = [None] * NT
        tu_box = {}

        def tu_ap(t, off, dims):
            return bass.AP(t.tensor, t[:].offset + off, [[TUP, P]] + dims)

        # ================= phase 1 step =================
        # tu layout per partition (c,ho): feat hr*72 + j*8 + k:
        #   [T|u][row k, col j] for heads hr. ag layout: (j, hr, i, k):
        #   j*NF//8? -> j*(NHR*64) + hr*64 + i*8 + k, A row-major (i, k).
        # vg layout: (j, hr, i): j*HRI + hr*8 + i.
        AGF = NHR * BD * BD  # 256 per j-slice

        def phase1_init(ag, vg, jr):
            tu = tupool.tile([P, TUP], FP16, tag="tu", name="tu_init")
            # Tu[k row, j col] = A[k, j]: out (hr, j, k) = (72, 8, 1);
            # A row-major feat hr*64 + k*8 + j -> in strides (64, 1, 8)
            nc.vector.tensor_copy(
                tu_ap(tu, 0, [[BD * JW, NHR], [BD, BD], [1, BD]]),
                bass.AP(ag.tensor, ag[:].offset + jr * AGF,
                        [[8 * AGF, P], [BD * BD, NHR], [1, BD], [BD, BD]]))
            # u := v_0
            nc.vector.tensor_copy(
                tu_ap(tu, BD * BD, [[BD * JW, NHR], [1, BD]]),
                bass.AP(vg.tensor, vg[:].offset + jr * HRI,
                        [[8 * HRI, P], [BD, NHR], [1, BD]]))
            tu_box['tu'] = tu

        def phase1_step(ag, vg, jr, name):
            tu = tu_box['tu']
            agb = ag[:].offset + jr * AGF
            # mo[hr][j, i, k] = A[i, k] * Tu[k, j]
            mo = mopool.tile([P, TUP * BD], FP16, tag="mo", name=f"mo{name}")
            MOF = TUP * BD  # 2304
            for hr in range(NHR):
                nc.vector.tensor_tensor(
                    bass.AP(mo.tensor, mo[:].offset + hr * JW * BD * BD,
                            [[MOF, P], [BD * BD, JW], [BD, BD], [1, BD]]),
                    bass.AP(ag.tensor, agb + hr * BD * BD,
                            [[8 * AGF, P], [0, JW], [BD, BD], [1, BD]]),
                    tu_ap(tu, hr * JW * BD,
                          [[BD, JW], [0, BD], [1, BD]]),
                    ALU.mult)
            # sum over k: 3 adds (kk-minor layouts)
            s1 = s1pool.tile([P, TUP * 4], FP16, tag="s1", name=f"s1{name}")
            S1F = TUP * 4
            nc.vector.tensor_tensor(
                bass.AP(s1.tensor, s1[:].offset, [[S1F, P], [32, 36], [4, BD], [1, 4]]),
                bass.AP(mo.tensor, mo[:].offset, [[MOF, P], [64, 36], [8, BD], [1, 4]]),
                bass.AP(mo.tensor, mo[:].offset + 4, [[MOF, P], [64, 36], [8, BD], [1, 4]]),
                ALU.add)
            s2 = s2pool.tile([P, TUP * 2], FP16, tag="s2", name=f"s2{name}")
            S2F = TUP * 2
            nc.vector.tensor_tensor(
                bass.AP(s2.tensor, s2[:].offset, [[S2F, P], [16, 36], [2, BD], [1, 2]]),
                bass.AP(s1.tensor, s1[:].offset, [[S1F, P], [32, 36], [4, BD], [1, 2]]),
                bass.AP(s1.tensor, s1[:].offset + 2, [[S1F, P], [32, 36], [4, BD], [1, 2]]),
                ALU.add)
            tun = tupool.tile([P, TUP], FP16, tag="tu", name=f"tu{name}")
            nc.vector.tensor_tensor(
                tu_ap(tun, 0, [[BD, 36], [1, BD]]),
                bass.AP(s2.tensor, s2[:].offset, [[S2F, P], [16, 36], [2, BD]]),
                bass.AP(s2.tensor, s2[:].offset + 1, [[S2F, P], [16, 36], [2, BD]]),
                ALU.add)
            # u += v
            nc.vector.tensor_tensor(
                tu_ap(tun, BD * BD, [[JW * BD, NHR], [1, BD]]),
                tu_ap(tun, BD * BD, [[JW * BD, NHR], [1, BD]]),
                bass.AP(vg.tensor, vg[:].offset + jr * HRI,
                        [[8 * HRI, P], [BD, NHR], [1, BD]]),
                ALU.add)
            tu_box['tu'] = tun

        # ================= stage A (+ interleaved phase 1) =================
        for q in range(NQ):
            # xT pre-shuffled on host: col = tau*128 + c*8 + j
            xq = xpool.tile([P, 4, QT], BF16, tag="xq")
            for k in range(4):
                nc.sync.dma_start(
                    xq[:, k, :],
                    bass.AP(xT, k * P * TOK + q * QT, [[TOK, P], [1, QT]]))

            hid_t = hpool.tile([P, HID // P, QT], BF16, tag="hid")
            for mb in range(4):
                w1b = w1pool.tile([P, 4, 8 * P], BF16, tag="w1b",
                                  name=f"w1b{q}_{mb}")
                nc.sync.dma_start(
                    w1b[:], w1[:, bass.ts(mb, 8 * P)]
                    .rearrange("(k p) m -> p k m", p=P))
                for m8 in range(8):
                    m = mb * 8 + m8
                    ps = l1ps.tile([P, QT], F32, tag="l1")
                    for k in range(4):
                        nc.tensor.matmul(ps[:], w1b[:, k, bass.ts(m8, P)],
                                         xq[:, k, :], start=(k == 0), stop=(k == 3))
                    nc.scalar.activation(hid_t[:, m, :], ps[:], AF.Relu,
                                         bias=b1_s[:, m:m + 1])

            hv_t = hpool.tile([P, 4, QT], BF16, tag="hv")
            for m in range(4):
                ps = l1ps.tile([P, QT], F32, tag="l1")
                for k in range(4):
                    nc.tensor.matmul(ps[:], v1_s[:, k, bass.ts(m, P)], xq[:, k, :],
                                     start=(k == 0), stop=(k == 3))
                nc.scalar.activation(hv_t[:, m, :], ps[:], AF.Relu,
                                     bias=c1_s[:, m:m + 1])

            # ---- L2: token-major blk, W2 streamed per (n, k-group) ----
            blks = [blkpool.tile([P, NF], FP16, tag="blk", name=f"blk{q}_{i}")
                    for i in range(TPQ)]
            for n in range(NF // 512):
                pss = [l2ps.tile([P, 512], F32, tag="l2", name=f"l2ps{q}_{n}_{i}")
                       for i in range(TPQ)]
                for ttq in range(TPQ):
                    nc.tensor.matmul(pss[ttq][:], ones_s[:1, :],
                                     b2_s[:1, bass.ts(n, 512)], start=True, stop=False)
                for kg in range(HID // P // 4):
                    w2t = w2pool.tile([P, 4, 512], BF16, tag="w2t")
                    nc.sync.dma_start(
                        w2t[:], bass.AP(w2, (kg * 4 * P) * NF + n * 512,
                                        [[NF, P], [P * NF, 4], [1, 512]]))
                    for k4 in range(4):
                        k = kg * 4 + k4
                        for ttq in range(TPQ):
                            nc.tensor.matmul(pss[ttq][:], hid_t[:, k, bass.ts(ttq, P)],
                                             w2t[:, k4, :], start=False,
                                             stop=(k == HID // P - 1))
                for ttq in range(TPQ):
                    nc.scalar.activation(blks[ttq][:, bass.ts(n, 512)], pss[ttq][:],
                                         AF.Identity)

            # ---- per tok-tile: v2, normalization, scan feed ----
            for ttq in range(TPQ):
                tau = q * TPQ + ttq

                psv = vps.tile([P, VF], F32, tag="v")
                nc.tensor.matmul(psv[:], ones_s[:1, :], c2_s[:1, :],
                                 start=True, stop=False)
                for k in range(4):
                    nc.tensor.matmul(psv[:], hv_t[:, k, bass.ts(ttq, P)],
                                     v2_s[:, k, :], start=False, stop=(k == 3))
                vt = vtpool.tile([P, VF], FP16, tag="vt")
                nc.scalar.activation(vt[:], psv[:], AF.Identity)
                # scratch row = tau*128 + j*16 + c  (j-major)
                nc.sync.dma_start(
                    bass.AP(v_dram, tau * P * VF,
                            [[VF, K], [K * VF, 8], [1, VF]]),
                    vt[:])

                blk = blks[ttq]
                pw = sqpool.tile([P, NF], FP16, tag="pw")
                nc.scalar.activation(pw[:], blk[:], AF.Square)
                nc.scalar.activation(pw[:], pw[:], AF.Ln)
                nc.scalar.activation(pw[:], pw[:], AF.Exp, scale=0.6)
                # pst[h, k] = sum_i |blk|^1.2 ; feat (h, i, k)
                pst = smpool.tile([P, HL * BD], F32, tag="pst")
                nc.vector.tensor_reduce(
                    bass.AP(pst.tensor, pst[:].offset, [[HL * BD, P], [BD, HL], [1, BD]]),
                    bass.AP(pw.tensor, pw[:].offset,
                            [[NF, P], [64, HL], [1, BD], [BD, BD]]),
                    axis=AX.X, op=ALU.add)
                nc.scalar.activation(pst[:], pst[:], AF.Ln)
                nc.scalar.activation(pst[:], pst[:], AF.Exp, scale=1.0 / 1.2)
                dm = smpool.tile([P, HL], F32, tag="dm")
                nc.vector.tensor_reduce(
                    dm[:].rearrange("p (h one) -> p h one", h=HL, one=1),
                    pst[:].rearrange("p (h k) -> p h k", h=HL, k=BD),
                    axis=AX.X, op=ALU.max)
                rc = smpool.tile([P, HL], FP16, tag="rc")
                nc.vector.reciprocal(rc[:], dm[:])
                rcr = smpool.tile([P, HL * BD], FP16, tag="rcr")
                nc.vector.tensor_copy(
                    bass.AP(rcr.tensor, rcr[:].offset, [[HL * BD, P], [BD, HL], [1, BD]]),
                    bass.AP(rc.tensor, rc[:].offset, [[HL, P], [1, HL], [0, BD]]))
                # A = blk * rc : natural row-major (h, i, k), all fp16 packed
                aT = apool.tile([P, NF], FP16, tag="aT")
                nc.vector.tensor_tensor(
                    bass.AP(aT.tensor, aT[:].offset,
                            [[NF, P], [64, HL], [BD, BD], [1, BD]]),
                    bass.AP(blk.tensor, blk[:].offset,
                            [[NF, P], [64, HL], [BD, BD], [1, BD]]),
                    bass.AP(rcr.tensor, rcr[:].offset,
                            [[HL * BD, P], [BD, HL], [0, BD], [1, BD]]),
                    ALU.mult)
                nc.sync.dma_start(
                    bass.AP(a_dram, tau * P * NF,
                            [[NF, K], [K * NF, 8], [1, NF]]),
                    aT[:])

                # redistribute (c,j) -> (c,ho): gather A and v for this tile.
                # src row (tau, j, c): merged (c,ho) stride = AGF
                ag = agpool.tile([P, 8 * AGF], FP16, tag="ag", name=f"ag{tau}")
                nc.sync.dma_start(ag[:], bass.AP(
                    a_dram, tau * P * NF,
                    [[AGF, P], [K * NF, 8], [1, AGF]]))
                vg = vgpool.tile([P, 8 * HRI], FP16, tag="vg", name=f"vg{tau}")
                nc.sync.dma_start(vg[:], bass.AP(
                    v_dram, tau * P * VF,
                    [[HRI, P], [K * VF, 8], [1, HRI]]))
                vg_tiles[tau] = vg

                for jr in range(8):
                    r = tau * 8 + jr
                    if r == 0:
                        phase1_init(ag, vg, jr)
                    else:
                        phase1_step(ag, vg, jr, f"{r}")

        # ================= phase B: combine chunk transitions =================
        tu = tu_box['tu']
        tuB = scpool.tile([NHO, K * TUP], FP16, tag="tuB")
        for c in range(K):
            nc.sync.dma_start(tuB[:, c * TUP:(c + 1) * TUP],
                              tu[c * NHO:(c + 1) * NHO, :])
        s_seq = scpool.tile([NHO, (K + 1) * HRI], FP16, tag="sseq")
        nc.vector.tensor_copy(s_seq[:, 0:HRI], a0_s[:])
        SSF = (K + 1) * HRI
        TBF = K * TUP
        for c in range(K):
            # s_rep[hr, col, row] = s[hr, col]
            srep = smpool.tile([NHO, TUP - HRI], FP16, tag="srep",
                               name=f"srep{c}")
            SRF = TUP - HRI  # 256: (hr, col8, row8)
            nc.vector.tensor_copy(
                bass.AP(srep.tensor, srep[:].offset,
                        [[SRF, NHO], [64, NHR], [8, BD], [1, BD]]),
                bass.AP(s_seq.tensor, s_seq[:].offset + c * HRI,
                        [[SSF, NHO], [BD, NHR], [1, BD], [0, BD]]))
            # mo[hr, col, row] = T[row, col] * s[col]
            moB = pcpool.tile([NHO, SRF], FP16, tag="moB", name=f"moB{c}")
            nc.vector.tensor_tensor(
                bass.AP(moB.tensor, moB[:].offset,
                        [[SRF, NHO], [64, NHR], [8, BD], [1, BD]]),
                bass.AP(tuB.tensor, tuB[:].offset + c * TUP,
                        [[TBF, NHO], [JW * BD, NHR], [BD, BD], [1, BD]]),
                bass.AP(srep.tensor, srep[:].offset,
                        [[SRF, NHO], [64, NHR], [8, BD], [1, BD]]),
                ALU.mult)
            # reduce over col, add u
            sred = smpool.tile([NHO, HRI], FP16, tag="sred", name=f"sred{c}")
            nc.vector.tensor_reduce(
                bass.AP(sred.tensor, sred[:].offset, [[HRI, NHO], [BD, NHR], [1, BD]]),
                bass.AP(moB.tensor, moB[:].offset,
                        [[SRF, NHO], [64, NHR], [1, BD], [8, BD]]),
                axis=AX.X, op=ALU.add)
            nc.vector.tensor_tensor(
                bass.AP(s_seq.tensor, s_seq[:].offset + (c + 1) * HRI,
                        [[SSF, NHO], [BD, NHR], [1, BD]]),
                bass.AP(sred.tensor, sred[:].offset, [[HRI, NHO], [BD, NHR], [1, BD]]),
                bass.AP(tuB.tensor, tuB[:].offset + c * TUP + BD * BD,
                        [[TBF, NHO], [JW * BD, NHR], [1, BD]]),
                ALU.add)
        # chunk-start states back to (c, ho) partitions
        s_init = scpool.tile([P, HRI], FP16, tag="sinit")
        for c in range(K):
            nc.sync.dma_start(s_init[c * NHO:(c + 1) * NHO, :],
                              s_seq[:, c * HRI:(c + 1) * HRI])

        # ================= phase C: re-scan with true inits =================
        s_outF = scpool.tile([P, C * HRI], FP16, tag="soutF")
        SOF = C * HRI

        def sout_ap(r, dims):
            return bass.AP(s_outF.tensor, s_outF[:].offset + r * HRI,
                           [[SOF, P]] + dims)

        for tau in range(NT):
            # re-gather A for this tile (double-buffered prefetch)
            ag2 = agpool.tile([P, 8 * AGF], FP16, tag="ag", name=f"ag2_{tau}")
            nc.sync.dma_start(ag2[:], bass.AP(
                a_dram, tau * P * NF,
                [[AGF, P], [K * NF, 8], [1, AGF]]))

            vg = vg_tiles[tau]
            for jr in range(8):
                r = tau * 8 + jr
                sprev = (bass.AP(s_init.tensor, s_init[:].offset,
                                 [[HRI, P], [BD, NHR], [0, BD], [1, BD]])
                         if r == 0 else
                         sout_ap(r - 1, [[BD, NHR], [0, BD], [1, BD]]))
                # mo2[hr, i, k] = A[i, k] * s_prev[k]
                mo2 = pcpool.tile([P, HRI * BD], FP16, tag="mo2", name=f"mo2_{r}")
                nc.vector.tensor_tensor(
                    bass.AP(mo2.tensor, mo2[:].offset,
                            [[HRI * BD, P], [BD * BD, NHR], [BD, BD], [1, BD]]),
                    bass.AP(ag2.tensor, ag2[:].offset + jr * AGF,
                            [[8 * AGF, P], [BD * BD, NHR], [BD, BD], [1, BD]]),
                    sprev, ALU.mult)
                sred = s2pool.tile([P, HRI], FP16, tag="sredc", name=f"sredc{r}")
                nc.vector.tensor_reduce(
                    bass.AP(sred.tensor, sred[:].offset, [[HRI, P], [1, HRI]]),
                    bass.AP(mo2.tensor, mo2[:].offset,
                            [[HRI * BD, P], [BD, HRI], [1, BD]]),
                    axis=AX.X, op=ALU.add)
                nc.vector.tensor_tensor(
                    sout_ap(r, [[1, HRI]]),
                    bass.AP(sred.tensor, sred[:].offset, [[HRI, P], [1, HRI]]),
                    bass.AP(vg.tensor, vg[:].offset + jr * HRI,
                            [[8 * HRI, P], [1, HRI]]),
                    ALU.add)

            # stream out this tile in scan order; host reorders rows.
            # out row tau*128 + (c*8+ho), col (jr, hr, i)
            s32 = pcpool.tile([P, 8 * HRI], F32, tag="s32", name=f"s32_{tau}")
            nc.scalar.activation(
                s32[:], bass.AP(s_outF.tensor, s_outF[:].offset + tau * 8 * HRI,
                                [[SOF, P], [1, 8 * HRI]]),
                AF.Identity)
            nc.sync.dma_start(out[bass.ds(tau * P, P), :], s32[:])

    nc.compile()
    return nc


# ---------------- host side ----------------

_NC_CACHE = {}


def _get_nc(TOK=SEQ):
    if TOK not in _NC_CACHE:
        _NC_CACHE[TOK] = build_nc(TOK=TOK)
    return _NC_CACHE[TOK]


def prep_shared(W1, b1, W2, b2, V1, c1, V2, c2, a0):
    bf = ml_dtypes.bfloat16
    W2r = W2.reshape(H, BD, BD, HID)
    W2c = (W2r - W2r.mean(axis=1, keepdims=True)).reshape(H * BD * BD, HID)
    b2r = b2.reshape(H, BD, BD)
    b2c = (b2r - b2r.mean(axis=1, keepdims=True)).reshape(-1)
    shared = {
        "w1": np.ascontiguousarray(W1.T).astype(bf),
        "b1": np.asarray(b1).reshape(HID, 1).astype(np.float32),
        "v1": np.ascontiguousarray(V1.T).astype(bf),
        "c1": np.asarray(c1).reshape(EMB, 1).astype(np.float32),
    }
    halves = []
    for half in range(2):
        rsl = slice(half * NF, (half + 1) * NF)
        vsl = slice(half * VF, (half + 1) * VF)
        hsl = slice(half * HL, (half + 1) * HL)
        a0h = np.asarray(a0)[0, hsl]                       # [32, 8]
        a0p = a0h.reshape(NHO, NHR * BD)                   # [ho, (hr, i)]
        halves.append({
            "w2": np.ascontiguousarray(W2c[rsl].T).astype(bf),
            "b2": b2c[rsl].reshape(1, NF).astype(bf),
            "v2": np.ascontiguousarray(V2[vsl].T).astype(bf),
            "c2": np.asarray(c2)[vsl].reshape(1, VF).astype(bf),
            "a0": a0p.astype(ml_dtypes.float16 if hasattr(ml_dtypes, 'float16')
                             else np.float16),
        })
    return shared, halves


def make_in_maps(x, W1, b1, W2, b2, V1, c1, V2, c2, a0):
    shared, halves = prep_shared(W1, b1, W2, b2, V1, c1, V2, c2, a0)
    bf = ml_dtypes.bfloat16
    in_maps = []
    for core in range(N_CORES):
        b, half = core // 2, core % 2
        m = dict(shared)
        m.update(halves[half])
        # column order (tau, c, j): col = tau*128 + c*8 + j for
        # token t = c*128 + tau*8 + j
        xs = np.asarray(x)[b].T.reshape(EMB, K, SEQ // P, 8)
        m["xT"] = np.ascontiguousarray(
            xs.transpose(0, 2, 1, 3).reshape(EMB, SEQ)).astype(bf)
        in_maps.append(m)
    return in_maps


def kernel(x, W1, b1, W2, b2, V1, c1, V2, c2, a0):
    from concourse import bass_utils
    nc = _get_nc(SEQ)
    in_maps = make_in_maps(x, W1, b1, W2, b2, V1, c1, V2, c2, a0)
    res = bass_utils.run_bass_kernel_spmd(nc, in_maps, core_ids=list(range(N_CORES)))
    out = np.zeros((BS, SEQ, EMB), np.float32)
    for core in range(N_CORES):
        b, half = core // 2, core % 2
        # kernel row = tau*128 + c*8 + ho, col = jr*32 + hr*8 + i
        # true row t = c*128 + tau*8 + jr, col = ho*32 + hr*8 + i
        r = res.results[core]["out"].reshape(SEQ // P, K, NHO, 8, NHR, BD)
        r = r.transpose(1, 0, 3, 2, 4, 5).reshape(SEQ, VF)
        out[b, :, half * VF:(half + 1) * VF] = r
    return out
